# revision 8
# baseline (speedup 1.0000x reference)
"""Sharded causal attention (decode-append) kernel for 8 NeuronCores.

Problem: 32 heads x 128 head_size, seq_len=512 new tokens appended at
offset=3584 into a 4096-entry KV cache. Head-parallel sharding: core c
owns heads 4c..4c+3 (contiguous 512-column slices of every tensor).

Host-side prep (inside kernel()): Q^T and K^T are pre-transposed per
head and cast to fp16; V is pre-packed into [pair, superchunk] SBUF
tiles. All matmul accumulation is fp32 in PSUM; output is DMAed out
as fp16 (well within the error budget) and widened on the host.

Per-core kernel (Tile framework): each head's context is processed as
13 exp-groups sized to PSUM tiles A=[128,1536] / B=[128,1024] in
strict alternation (3/2 prefix blocks per group; the 4 causal-masked
diagonal blocks are packed into one B group and one A group
mid-stream). Stages are emitted as a software pipeline with lag:
  slot i:   QK matmuls of group i (PE) + exp of group i (ScalarE)
  slot i+1: AV matmuls + 2:1/3:1 folds of group i-1
  slot i+2: SUM matmul of group i-2 (+ per-head epilogue)
so the in-order PE queue always has ready QK work ahead of the
exp-dependent AV work: PE never idles on ScalarE, keeping the HAM
clock-gate at 2.4 GHz. ~20 dummy matmuls on a const tile pre-warm the
clock during the initial DMA wait. Triangle masks are accumulated
into the score PSUM by identity-matmuls of a baked [-60000|0] fp16
matrix (PE), keeping the QK->exp chain off the VectorE queue.

Teardown is a patched lean version of Tile's drain (single all-engine
barrier, semaphore clears split across engines).
"""

import sys

if "/opt/trn_rl_repo" not in sys.path:
    sys.path.insert(0, "/opt/trn_rl_repo")

import ml_dtypes
import numpy as np

NUM_HEADS = 32
HEAD = 128
HIDDEN = NUM_HEADS * HEAD
MAX_SEQ = 4096
N_CORES = 8
HEADS_PER_CORE = NUM_HEADS // N_CORES          # 4
CW = HEADS_PER_CORE * HEAD                     # 512 columns per core
SEQ = 512                                      # seq_len
OFFSET = 3584                                  # cache offset
CTX = OFFSET + SEQ                             # 4096 context length
TBLK = 128                                     # context block
NTB = CTX // TBLK                              # 32 t-blocks
PREFIX_TB = OFFSET // TBLK                     # 28 unmasked blocks
SCALE = float(1.0 / np.sqrt(np.float32(HEAD)))
MASK_NEG = -60000.0                            # fp16-representable; exp -> 0

# per-head group schedule: 13 slots alternating PSUM tile A (3 banks,
# up to 3 blocks) and B (2 banks, up to 2 blocks). 'd1' = diag blocks
# 28,29 (B tile), 'd2' = diag blocks 30,31 (A tile), placed mid-stream.
# prefix groups take consecutive blocks 0..27.
GROUP_SIZES = [3, 2, 3, 2, 3, "d1", "d2", 2, 3, 2, 3, 2, 3]

_CACHE: dict = {}


def _build():
    import concourse.bacc as bacc
    import concourse.tile as tile
    from concourse import mybir
    from concourse.vector_clock import ScopedClock

    def _lean_drain_and_barrier(self, tick_clock, wait_clock):
        # Stock teardown: drain + barrier + serial gpsimd sem-clear + barrier
        # (~12us). Here: drain + one barrier, then the sem-clears split
        # round-robin across all five engines (~5x faster wall-clock).
        from concourse._compat import exact_div  # noqa: F401

        nc = self.nc
        drain_inst = nc.sync.drain()
        wait_clock.add_sem_waits(
            drain_inst.ins, ScopedClock({None: tick_clock.global_clock}))
        nc.all_engine_barrier()
        popped = nc._tile_sem_poison_stack.pop()
        assert popped is self._sem_poison

        sems = list(self.sems.allocated().values())
        sem_nums = sorted(s.num if hasattr(s, "num") else s for s in sems)
        engines = [nc.gpsimd, nc.vector, nc.scalar, nc.tensor, nc.sync]
        ranges = []
        start = prev = None
        for n in sem_nums:
            if prev is None or n != prev + 1:
                if prev is not None:
                    ranges.append(range(start, prev + 1))
                start = n
            prev = n
        if prev is not None:
            ranges.append(range(start, prev + 1))
        for r in ranges:
            nc.gpsimd.dma_reset(r)
        chunks = []
        for r in ranges:
            vals = list(r)
            k = max(1, len(vals) // len(engines) + 1)
            for i in range(0, len(vals), k):
                seg = vals[i:i + k]
                chunks.append(range(seg[0], seg[-1] + 1))
        for i, r in enumerate(chunks):
            engines[i % len(engines)].sem_clear(r)
        nc._state.prepend_free_semaphores(sem_nums)
        for poison_set in nc._tile_sem_poison_stack:
            poison_set.update(sem_nums)

    tile.TileContext._drain_and_barrier = _lean_drain_and_barrier

    # min-pop sem allocator: denser sem-ID reuse -> far fewer distinct sems
    # to clear in the teardown.
    import concourse.bass as _bassmod
    _bassmod.is_customcomms_rdh_enabled = lambda: True

    F32 = mybir.dt.float32
    F16 = mybir.dt.float16
    EXP = mybir.ActivationFunctionType.Exp

    nc = bacc.Bacc()
    qt_d = nc.dram_tensor("qt", [HEADS_PER_CORE, 128, SEQ], F16,
                          kind="ExternalInput")
    kt_d = nc.dram_tensor("kt", [HEADS_PER_CORE, 128, CTX], F16,
                          kind="ExternalInput")
    # V packed per (pair, superchunk of 8 t-blocks): [128, 8*256]
    vp_d = nc.dram_tensor("vp", [HEADS_PER_CORE // 2 * (NTB // 8), 128, 2048],
                          F16, kind="ExternalInput")
    # packed consts: cols [0:128)=ones, [128:256)=identity, [256:384)=mask
    cst_d = nc.dram_tensor("cst", [128, 384], F16, kind="ExternalInput")
    out_d = nc.dram_tensor("outt", [HEADS_PER_CORE, 128, SEQ], F16,
                           kind="ExternalOutput")

    PW = 2 * HEAD      # 256 columns = one head-pair (for V tiles)
    NSC = NTB // 8     # 4 superchunks per head

    # resolve per-head group schedule -> block lists
    def head_groups():
        gs = []
        b = 0
        for s in GROUP_SIZES:
            if s == "d1":
                gs.append(dict(kind="d1"))
            elif s == "d2":
                gs.append(dict(kind="d2"))
            else:
                gs.append(dict(kind="p", blocks=list(range(b, b + s))))
                b += s
        assert b == PREFIX_TB
        return gs

    with tile.TileContext(nc) as tc:
        with (
            tc.tile_pool(name="consts", bufs=1) as consts,
            tc.tile_pool(name="qpool", bufs=4) as qpool,
            tc.tile_pool(name="kt0a", bufs=1) as kt0a_p,
            tc.tile_pool(name="kt0b", bufs=1) as kt0b_p,
            tc.tile_pool(name="kt0c", bufs=1) as kt0c_p,
            tc.tile_pool(name="ktp", bufs=2) as ktp,
            tc.tile_pool(name="ktd", bufs=2) as ktdp,
            tc.tile_pool(name="vpool", bufs=8) as vpool,
            tc.tile_pool(name="epool", bufs=5) as epool,
            tc.tile_pool(name="fold", bufs=4) as foldp,
            tc.tile_pool(name="f2p", bufs=2) as f2p,
            tc.tile_pool(name="fin", bufs=4) as fin,
            tc.tile_pool(name="psA", bufs=1, space="PSUM") as psA,
            tc.tile_pool(name="psB", bufs=1, space="PSUM") as psB,
            tc.tile_pool(name="psav", bufs=2, space="PSUM") as psav,
            tc.tile_pool(name="pssum", bufs=1, space="PSUM") as pssum,
        ):
            # ---------------- startup DMAs -----------------------------
            # consts first (warm-up matmuls need them), then qT (gpsimd q)
            cst = consts.tile([128, 384], F16, tag="cst")
            nc.gpsimd.dma_start(cst[:], cst_d[:])
            ones = cst[:, 0:128]
            ident = cst[:, 128:256]
            mask16 = cst[:, 256:384]
            qT = []
            for h in range(HEADS_PER_CORE):
                t = qpool.tile([128, SEQ], F16, tag=f"qT{h}", name=f"qT{h}")
                nc.gpsimd.dma_start(t[:], qt_d[h])
                qT.append(t)

            # head-0 kt prefix in 3 segments so the first groups start
            # before the whole prefix lands; heads 1-3 as one DMA each.
            # segment col ranges (within [0, 3584)):
            SEG0 = (0, 384)        # group s0 (blocks 0-2)
            SEG1 = (384, 1664)     # blocks 3-12 (groups s1-s4)
            SEG2 = (1664, 3584)    # blocks 13-27 (groups s7-s12)
            kt_tiles = {}          # h -> dict(seg tiles / full tile, diag)

            def load_kt_head0():
                a = kt0a_p.tile([128, SEG0[1] - SEG0[0]], F16, tag="k0a")
                nc.sync.dma_start(a[:], kt_d[0, :, SEG0[0]:SEG0[1]])
                kt_tiles[0] = {"segs": [a, None, None], "diag": None}

            def load_kt_head0_rest():
                b = kt0b_p.tile([128, SEG1[1] - SEG1[0]], F16, tag="k0b")
                nc.sync.dma_start(b[:], kt_d[0, :, SEG1[0]:SEG1[1]])
                kt_tiles[0]["segs"][1] = b
                d = ktdp.tile([128, SEQ], F16, tag="ktd", name="ktd0")
                nc.sync.dma_start(d[:], kt_d[0, :, OFFSET:CTX])
                kt_tiles[0]["diag"] = d
                c = kt0c_p.tile([128, SEG2[1] - SEG2[0]], F16, tag="k0c")
                nc.sync.dma_start(c[:], kt_d[0, :, SEG2[0]:SEG2[1]])
                kt_tiles[0]["segs"][2] = c

            def load_kt(h):
                if h >= HEADS_PER_CORE or h in kt_tiles:
                    return
                t = ktp.tile([128, OFFSET], F16, tag="ktp", name=f"ktp{h}")
                nc.sync.dma_start(t[:], kt_d[h, :, 0:OFFSET])
                d = ktdp.tile([128, SEQ], F16, tag="ktd", name=f"ktd{h}")
                nc.sync.dma_start(d[:], kt_d[h, :, OFFSET:CTX])
                kt_tiles[h] = {"full": t, "diag": d}

            def kt_block(h, b):
                """SBUF slice holding prefix t-block b of head h."""
                info = kt_tiles[h]
                if "full" in info:
                    return info["full"][:, b * 128:(b + 1) * 128]
                col = b * 128
                for si, (lo, hi) in enumerate((SEG0, SEG1, SEG2)):
                    if lo <= col < hi:
                        t = info["segs"][si]
                        return t[:, col - lo:col - lo + 128]
                raise AssertionError(b)

            v_tiles = {}           # (pair, sc) -> tile

            def load_v_sc(p, sc):
                if p >= HEADS_PER_CORE // 2 or (p, sc) in v_tiles:
                    return
                t = vpool.tile([128, 2048], F16, tag="vsc",
                               name=f"v{p}_{sc}")
                nc.sync.dma_start(t[:], vp_d[p * NSC + sc])
                v_tiles[(p, sc)] = t

            def load_v_pair(p):
                for sc in range(NSC):
                    load_v_sc(p, sc)

            def v_block(h, b):
                """SBUF slice of V t-block b (context block) for head h."""
                p, hh = h // 2, h % 2
                t = v_tiles[(p, b // 8)]
                col = (b % 8) * PW + hh * 128
                return t[:, col:col + 128]

            # startup issue order tuned to first-use times: the first AV
            # needs V sc0 at ~+2us, QK(s1) needs the second kt segment,
            # the diag groups (slots 5-6) need ktd0 + V sc3.
            load_kt_head0()
            load_v_sc(0, 0)
            load_kt_head0_rest()
            load_v_sc(0, 1)
            load_v_sc(0, 3)
            load_v_sc(0, 2)
            load_kt(1)

            # ---------------- PE clock warm-up -------------------------
            # ~20 junk matmuls on the const tile while kt/qt DMAs land:
            # keeps the HAM activity window busy so the first real QK
            # already runs at 2.4 GHz. Results are overwritten.
            warm = psA.tile([128, 1536], F32, tag="scA", name="warm")
            for w in range(20):
                nc.tensor.matmul(warm[:, 0:128], ident, ones,
                                 start=True, stop=True)

            # ---------------- flattened group list ----------------------
            groups = []
            for h in range(HEADS_PER_CORE):
                for si, g in enumerate(head_groups()):
                    g = dict(g)
                    g["h"] = h
                    g["slot"] = si
                    groups.append(g)
            for idx, g in enumerate(groups):
                g["idx"] = idx
            last_of_head = {g["h"]: g["idx"] for g in groups}

            head_state: dict = {}

            def get_out_ps(h):
                st = head_state.setdefault(h, {})
                if "out_ps" not in st:
                    st["out_ps"] = psav.tile([128, SEQ], F32, tag="avacc",
                                             name=f"avacc{h}")
                return st["out_ps"]

            def get_sum_ps(h):
                st = head_state.setdefault(h, {})
                if "sum_ps" not in st:
                    st["sum_ps"] = pssum.tile([128, SEQ], F32, tag="sumacc",
                                              name=f"sumacc{h}")
                return st["sum_ps"]

            # diag geometry: block k covers s in [128k, 512) => n=512-128k
            # d1 packs k=0 at col 0 (n=512), k=1 at col 512 (n=384)
            # d2 packs k=2 at col 0 (n=256), k=3 at col 256 (n=128)
            DCOLS = {(0, "off"): 0, (1, "off"): 512,
                     (2, "off"): 0, (3, "off"): 256}

            def stage_a(g):
                """QK matmuls into PSUM (+ diag masks) + exp."""
                h, si = g["h"], g["slot"]
                if si == 0:
                    # prefetch: next head's kt; next pair's V at odd heads
                    load_kt(h + 1)
                    if h % 2 == 1:
                        load_v_pair(h // 2 + 1)
                kind = g["kind"]
                use_a = (si % 2 == 0)
                pool = psA if use_a else psB
                width = 1536 if use_a else 1024
                sc = pool.tile([128, width], F32,
                               tag="scA" if use_a else "scB",
                               name=f"sc{h}_{si}")
                e = epool.tile([128, 1536], F16, tag="e", name=f"e{g['idx']}")
                if kind == "p":
                    blocks = g["blocks"]
                    n = len(blocks) * 512
                    for jj, b in enumerate(blocks):
                        nc.tensor.matmul(
                            sc[:, jj * 512:(jj + 1) * 512],
                            kt_block(h, b), qT[h][:],
                            start=True, stop=True)
                    nc.scalar.activation(e[:, 0:n], sc[:, 0:n],
                                         EXP, scale=SCALE)
                    g["n"] = n
                else:
                    kd = kt_tiles[h]["diag"]
                    ks = (0, 1) if kind == "d1" else (2, 3)
                    for k in ks:
                        off = 128 * k          # s-range start
                        n = SEQ - off
                        col = DCOLS[(k, "off")]
                        # mask first, QK accumulates onto it. d1's two
                        # blocks land in separate PSUM banks (start=True
                        # clears a bank); d2's share bank 0, so only the
                        # first write clears and only the last stops.
                        if kind == "d1":
                            m_start, q_stop = True, True
                        else:
                            m_start, q_stop = (k == 2), (k == 3)
                        nc.tensor.matmul(
                            sc[:, col:col + 128], ident, mask16,
                            start=m_start, stop=False)
                        nc.tensor.matmul(
                            sc[:, col:col + n],
                            kd[:, k * 128:(k + 1) * 128],
                            qT[h][:, off:SEQ],
                            start=False, stop=q_stop)
                    tot = (512 + 384) if kind == "d1" else (256 + 128)
                    nc.scalar.activation(e[:, 0:tot], sc[:, 0:tot],
                                         EXP, scale=SCALE)
                    g["n"] = tot
                g["e"] = e

            def stage_b(g):
                """AV matmuls (+ folds for prefix groups)."""
                h = g["h"]
                out_ps = get_out_ps(h)
                e = g["e"]
                kind = g["kind"]
                if kind == "p":
                    blocks = g["blocks"]
                    for jj, b in enumerate(blocks):
                        nc.tensor.matmul(
                            out_ps[:], v_block(h, b),
                            e[:, jj * 512:(jj + 1) * 512],
                            start=(b == 0),
                            stop=(b == PREFIX_TB - 1))
                    if len(blocks) == 3:
                        f = foldp.tile([128, 512], F16, tag="f",
                                       name=f"f{g['idx']}")
                        nc.vector.tensor_add(f[:], e[:, 0:512],
                                             e[:, 512:1024])
                        f2 = f2p.tile([128, 512], F16, tag="f2",
                                      name=f"f2{g['idx']}")
                        nc.vector.tensor_add(f2[:], f[:], e[:, 1024:1536])
                        g["fsum"] = f2
                    else:
                        f = foldp.tile([128, 512], F16, tag="f",
                                       name=f"f{g['idx']}")
                        nc.vector.tensor_add(f[:], e[:, 0:512],
                                             e[:, 512:1024])
                        g["fsum"] = f
                else:
                    ks = (0, 1) if kind == "d1" else (2, 3)
                    for k in ks:
                        off = 128 * k
                        n = SEQ - off
                        col = DCOLS[(k, "off")]
                        nc.tensor.matmul(
                            out_ps[:, off:SEQ],
                            v_block(h, PREFIX_TB + k),
                            e[:, col:col + n],
                            start=False, stop=False)

            def stage_c(g):
                """SUM matmul(s); epilogue at head end."""
                h, si = g["h"], g["slot"]
                sum_ps = get_sum_ps(h)
                if g["kind"] == "p":
                    nc.tensor.matmul(sum_ps[:], ones, g["fsum"][:],
                                     start=(si == 0),
                                     stop=(si == len(GROUP_SIZES) - 1))
                else:
                    e = g["e"]
                    ks = (0, 1) if g["kind"] == "d1" else (2, 3)
                    for k in ks:
                        off = 128 * k
                        n = SEQ - off
                        col = DCOLS[(k, "off")]
                        nc.tensor.matmul(
                            sum_ps[:, off:SEQ], ones, e[:, col:col + n],
                            start=False, stop=False)
                if g["idx"] == last_of_head[h]:
                    out_ps = get_out_ps(h)
                    recip = fin.tile([128, SEQ], F32, tag="recip",
                                     name=f"recip{h}")
                    nc.vector.reciprocal_approx_fast(recip[:], sum_ps[:])
                    outT = fin.tile([128, SEQ], F16, tag="outT",
                                    name=f"outT{h}")
                    nc.vector.tensor_mul(outT[:], out_ps[:], recip[:])
                    nc.sync.dma_start(out_d[h], outT[:])
                    head_state.pop(h, None)

            NG = len(groups)
            for i in range(NG + 2):
                if i < NG:
                    stage_a(groups[i])
                if 1 <= i <= NG:
                    stage_b(groups[i - 1])
                if 2 <= i <= NG + 1:
                    stage_c(groups[i - 2])

    nc.finalize()
    return nc


def _consts():
    ones = np.ones((128, 128), dtype=np.float16)
    ident = np.eye(128, dtype=np.float16)
    # triangle mask for the diagonal 128-blocks: allowed iff s' >= t
    s = np.arange(128)[None, :]
    t = np.arange(128)[:, None]
    mask16 = np.where(s >= t, 0.0, MASK_NEG).astype(np.float16)
    return np.concatenate([ones, ident, mask16], axis=1)  # [128, 384]


def _in_maps(query, key, value, kv_cache):
    bf = np.float16
    q_bf = query.astype(bf)                        # [512, 4096]
    k_full = np.concatenate([kv_cache[0, :OFFSET], key], axis=0)   # [4096, 4096]
    v_full = np.concatenate([kv_cache[1, :OFFSET], value], axis=0)
    k_bf = k_full.astype(bf)
    v_bf = v_full.astype(bf)

    cst = _consts()
    in_maps = []
    for c in range(N_CORES):
        cols = slice(c * CW, (c + 1) * CW)
        # [t, 4h*128] -> [4h, 128, t] transposed
        kt = np.ascontiguousarray(
            k_bf[:, cols].reshape(CTX, HEADS_PER_CORE, HEAD).transpose(1, 2, 0))
        qt = np.ascontiguousarray(
            q_bf[:, cols].reshape(SEQ, HEADS_PER_CORE, HEAD).transpose(1, 2, 0))
        # V packed per (pair, superchunk of 8 blocks): [p*4+sc, 128, 2048]
        vpk = (v_bf[:, cols]
               .reshape(4, 8, 128, 2, 256)        # [sc, b, part, pair, 256]
               .transpose(3, 0, 2, 1, 4)          # [pair, sc, part, b, 256]
               .reshape(2 * 4, 128, 2048))
        in_maps.append({
            "qt": qt,
            "kt": kt,
            "vp": np.ascontiguousarray(vpk),
            "cst": cst,
        })
    return in_maps


def kernel(query, key, value, kv_cache, offset, seq_len):
    query = np.asarray(query, dtype=np.float32)
    key = np.asarray(key, dtype=np.float32)
    value = np.asarray(value, dtype=np.float32)
    kv_cache = np.asarray(kv_cache, dtype=np.float32)
    assert int(offset) == OFFSET and int(seq_len) == SEQ, (offset, seq_len)

    if "nc" not in _CACHE:
        _CACHE["nc"] = _build()
    nc = _CACHE["nc"]

    from concourse.bass_utils import run_bass_kernel_spmd

    res = run_bass_kernel_spmd(nc, _in_maps(query, key, value, kv_cache),
                               list(range(N_CORES)))
    # outt[h, d, s] fp16 -> out[s, h*128+d] fp32, concatenated across cores
    outs = [np.ascontiguousarray(
                res.results[c]["outt"].astype(np.float32)
                .transpose(2, 0, 1).reshape(SEQ, CW))
            for c in range(N_CORES)]
    return np.concatenate(outs, axis=1)
